# revision 4
# baseline (speedup 1.0000x reference)
"""GQA prefill attention (12 Q heads / 2 KV heads, D=128, S=2048) on 8 TRN2 cores.

Sharding: balanced causal sequence-split. Core i owns query chunks (i, 15-i)
of 128 rows each, so every core covers 17 of the 136 causal key-blocks.
The paged-cache scatter+gather in the reference is an exact identity
(slot_mapping/block_table are built from the same permutation and the cache
starts empty with context_lens == S), so attention runs directly on the
freshly projected K/V in token order.

One SPMD program for all cores: per-core causal extents are handled by
host-provided multiplicative masks over uniform key loops (8 blocks for the
low chunk, 16 for the high chunk). rotate_half for RoPE runs on the tensor
engine via a constant +/-1 permutation matrix (engines cannot shift data
across partitions).
"""

from contextlib import ExitStack

import numpy as np
import ml_dtypes

import concourse.bass as bass
import concourse.mybir as mybir
import concourse.tile as tile
from concourse import bacc
from concourse import bass_utils

NQ, NKV, D = 12, 2, 128
G = NQ // NKV
HM = NQ * D            # 1536
S = 2048
CH = 128               # q chunk rows
NCHUNK = S // CH       # 16
NCORE = 8
KT = HM // 128         # 12 contraction tiles for the projections
NBLK = (8, 16)         # key-block loop bounds for (low, high) chunk
SCALE = 1.0 / float(np.sqrt(D))

BF16 = mybir.dt.bfloat16
FP32 = mybir.dt.float32
bfnp = ml_dtypes.bfloat16

_CACHE = {}


def build_kernel(ctx: ExitStack, tc: tile.TileContext, ins, outs):
    nc = tc.nc
    out_d = outs["out"]

    const = ctx.enter_context(tc.tile_pool(name="const", bufs=1))
    big = ctx.enter_context(tc.tile_pool(name="big", bufs=1))

    # ---- resident loads ----
    ident = const.tile([128, 128], BF16)
    nc.sync.dma_start(ident, ins["ident"])
    rotT = const.tile([128, 128], BF16)
    nc.sync.dma_start(rotT, ins["rotT"])
    bq_sb = const.tile([128, KT], FP32)
    nc.sync.dma_start(bq_sb, ins["bq"])
    bk_sb = const.tile([128, NKV], FP32)
    nc.sync.dma_start(bk_sb, ins["bk"])
    bv_sb = const.tile([128, 2 * D], FP32)
    nc.sync.dma_start(bv_sb, ins["bv"])
    cos_own = const.tile([128, 2 * CH], BF16)
    nc.sync.dma_start(cos_own, ins["cos_own"])
    sin_own = const.tile([128, 2 * CH], BF16)
    nc.sync.dma_start(sin_own, ins["sin_own"])
    cos_full = const.tile([128, S], BF16)
    nc.sync.dma_start(cos_full, ins["cos_full"])
    sin_full = const.tile([128, S], BF16)
    nc.sync.dma_start(sin_full, ins["sin_full"])
    mask_a = const.tile([128, NBLK[0] * 128], BF16)
    nc.sync.dma_start(mask_a, ins["mask_a"])
    mask_b = const.tile([128, NBLK[1] * 128], BF16)
    nc.sync.dma_start(mask_b, ins["mask_b"])
    masks = (mask_a, mask_b)

    WqT_sb = big.tile([128, KT, HM], BF16)
    nc.sync.dma_start(WqT_sb, ins["WqT"].rearrange("(a p) o -> p a o", p=128))
    WkT_sb = big.tile([128, KT, NKV * D], BF16)
    nc.sync.dma_start(WkT_sb, ins["WkT"].rearrange("(a p) o -> p a o", p=128))
    WvT_sb = big.tile([128, KT, NKV * D], BF16)
    nc.sync.dma_start(WvT_sb, ins["WvT"].rearrange("(a p) o -> p a o", p=128))
    xq_sb = big.tile([128, KT, 2 * CH], BF16)
    nc.sync.dma_start(xq_sb, ins["xT_own"].rearrange("(a p) t -> p a t", p=128))

    # persistent activations
    qTb = big.tile([128, NQ, 2 * CH], BF16)
    kTb = big.tile([128, NKV, S], BF16)
    v_sb = big.tile([128, NCHUNK, NKV * D], BF16)
    attn = big.tile([128, 2, NQ, 128], BF16)
    attnT = big.tile([128, NQ, 2 * CH], BF16)

    xT_d = ins["xT_full"].rearrange("(a p) t -> p a t", p=128)

    def rope(rp, psr, dst, src, cosf, sinf, T):
        """dst = src*cos + (R @ src)*sin; src/dst (128,T) bf16, cos/sin (128,T)."""
        rps = psr.tile([128, T], FP32, tag="rot")
        nc.tensor.matmul(rps, rotT, src, start=True, stop=True)
        t1 = rp.tile([128, T], BF16, tag="rope_t1")
        nc.vector.tensor_mul(t1, rps, sinf)
        t2 = rp.tile([128, T], BF16, tag="rope_t2")
        nc.vector.tensor_mul(t2, src, cosf)
        nc.vector.tensor_add(dst, t1, t2)

    # ---- phase B: projections + RoPE ----
    with tc.tile_pool(name="pB_ps", bufs=2, space="PSUM") as psB, \
         tc.tile_pool(name="pB_psr", bufs=2, space="PSUM") as psR, \
         tc.tile_pool(name="pB_x", bufs=2) as xs, \
         tc.tile_pool(name="pB_sb", bufs=2) as sbB:
        # k + v over all tokens, streaming xT in 512-token slabs
        for t4 in range(S // 512):
            sl = slice(t4 * 512, (t4 + 1) * 512)
            xc = xs.tile([128, KT, 512], BF16, tag="xslab")
            nc.sync.dma_start(xc, xT_d[:, :, sl])
            for ot in range(NKV):
                ps = psB.tile([128, 512], FP32, tag="psk")
                for kt in range(KT):
                    nc.tensor.matmul(
                        ps, WkT_sb[:, kt, ot * 128:(ot + 1) * 128], xc[:, kt, :],
                        start=(kt == 0), stop=(kt == KT - 1))
                kpre = sbB.tile([128, 512], BF16, tag="kpre")
                nc.scalar.activation(
                    kpre, ps, mybir.ActivationFunctionType.Identity,
                    bias=bk_sb[:, ot:ot + 1])
                rope(sbB, psR, kTb[:, ot, sl], kpre,
                     cos_full[:, sl], sin_full[:, sl], 512)
            for tt in range(4):
                ps = psB.tile([128, NKV * D], FP32, tag="psv")
                for kt in range(KT):
                    nc.tensor.matmul(
                        ps, xc[:, kt, tt * 128:(tt + 1) * 128], WvT_sb[:, kt, :],
                        start=(kt == 0), stop=(kt == KT - 1))
                nc.vector.tensor_add(v_sb[:, t4 * 4 + tt, :], ps, bv_sb)
        # q projection: out (o-tile 128, own 256 tokens), then RoPE per head
        for ot in range(NQ):
            ps = psB.tile([128, 2 * CH], FP32, tag="psq")
            for kt in range(KT):
                nc.tensor.matmul(
                    ps, WqT_sb[:, kt, ot * 128:(ot + 1) * 128], xq_sb[:, kt, :],
                    start=(kt == 0), stop=(kt == KT - 1))
            qpre = sbB.tile([128, 2 * CH], BF16, tag="qpre")
            nc.scalar.activation(
                qpre, ps, mybir.ActivationFunctionType.Identity,
                bias=bq_sb[:, ot:ot + 1])
            rope(sbB, psR, qTb[:, ot, :], qpre, cos_own, sin_own, 2 * CH)

    # ---- phase C: attention ----
    with tc.tile_pool(name="pC_s", bufs=2, space="PSUM") as ps_s, \
         tc.tile_pool(name="pC_t", bufs=3, space="PSUM") as ps_t, \
         tc.tile_pool(name="pC_o", bufs=2, space="PSUM") as ps_o, \
         tc.tile_pool(name="pC_p", bufs=2) as pp, \
         tc.tile_pool(name="pC_sm", bufs=4) as sm:
        for c in range(2):
            nblk = NBLK[c]
            nj4 = nblk // 4
            for h in range(NQ):
                g = h // G
                qT = qTb[:, h, c * CH:(c + 1) * CH]
                p_sb = pp.tile([128, NBLK[1] * 128], BF16, tag="p")
                sums = sm.tile([128, 4], FP32, tag="sums")
                for j4 in range(nj4):
                    sl = slice(j4 * 512, (j4 + 1) * 512)
                    ps = ps_s.tile([128, 512], FP32, tag="pss")
                    nc.tensor.matmul(ps, qT, kTb[:, g, sl], start=True, stop=True)
                    pe = pp.tile([128, 512], BF16, tag="pexp")
                    nc.scalar.activation(
                        pe, ps, mybir.ActivationFunctionType.Exp, scale=SCALE)
                    nc.vector.tensor_mul(p_sb[:, sl], pe, masks[c][:, sl])
                    nc.vector.reduce_sum(sums[:, j4:j4 + 1], p_sb[:, sl],
                                         axis=mybir.AxisListType.X)
                po = ps_o.tile([128, 128], FP32, tag="pso")
                for j in range(nblk):
                    pt_ps = ps_t.tile([128, 128], BF16, tag="pst")
                    nc.tensor.transpose(
                        pt_ps, p_sb[:, j * 128:(j + 1) * 128], ident)
                    pT = pp.tile([128, 128], BF16, tag="pT")
                    if j % 2 == 0:
                        nc.scalar.copy(pT, pt_ps)
                    else:
                        nc.vector.tensor_copy(pT, pt_ps)
                    nc.tensor.matmul(
                        po, pT, v_sb[:, j, g * D:(g + 1) * D],
                        start=(j == 0), stop=(j == nblk - 1))
                ssum = sm.tile([128, 1], FP32, tag="ssum")
                nc.vector.reduce_sum(ssum, sums[:, :nj4], axis=mybir.AxisListType.X)
                rsum = sm.tile([128, 1], FP32, tag="rsum")
                nc.vector.reciprocal(rsum, ssum)
                nc.scalar.mul(attn[:, c, h, :], po, rsum)

    # ---- phase D: transpose attn + output projection ----
    WoT_d = ins["WoT"].rearrange("(a p) m -> p a m", p=128)
    with tc.tile_pool(name="pD_t", bufs=3, space="PSUM") as ps_t2, \
         tc.tile_pool(name="pD_o", bufs=2, space="PSUM") as ps_oo, \
         tc.tile_pool(name="pD_w", bufs=2) as wo_s, \
         tc.tile_pool(name="pD_f", bufs=2) as fo:
        for c in range(2):
            for h in range(NQ):
                t_ps = ps_t2.tile([128, 128], BF16, tag="at")
                nc.tensor.transpose(t_ps, attn[:, c, h, :], ident)
                if h % 2 == 0:
                    nc.scalar.copy(attnT[:, h, c * CH:(c + 1) * CH], t_ps)
                else:
                    nc.vector.tensor_copy(attnT[:, h, c * CH:(c + 1) * CH], t_ps)
        for nb in range(HM // 512):
            wo = wo_s.tile([128, KT, 512], BF16, tag="wo")
            nc.sync.dma_start(wo, WoT_d[:, :, nb * 512:(nb + 1) * 512])
            for c in range(2):
                ps = ps_oo.tile([128, 512], FP32, tag="psoo")
                for ht in range(KT):
                    nc.tensor.matmul(
                        ps, attnT[:, ht, c * CH:(c + 1) * CH], wo[:, ht, :],
                        start=(ht == 0), stop=(ht == KT - 1))
                ob = fo.tile([128, 512], FP32, tag="ob")
                nc.scalar.copy(ob, ps)
                nc.sync.dma_start(
                    out_d[c * CH:(c + 1) * CH, nb * 512:(nb + 1) * 512], ob)


def build_program():
    nc = bacc.Bacc("TRN2", target_bir_lowering=False, debug=False,
                   enable_asserts=False, num_devices=NCORE)
    ins = {}

    def din(name, shape, dt=BF16):
        ins[name] = nc.dram_tensor(name, shape, dt, kind="ExternalInput").ap()

    din("xT_own", [HM, 2 * CH])
    din("xT_full", [HM, S])
    din("WqT", [HM, HM])
    din("WkT", [HM, NKV * D])
    din("WvT", [HM, NKV * D])
    din("WoT", [HM, HM])
    din("bq", [128, KT], FP32)
    din("bk", [128, NKV], FP32)
    din("bv", [128, NKV * D], FP32)
    din("cos_own", [128, 2 * CH])
    din("sin_own", [128, 2 * CH])
    din("cos_full", [128, S])
    din("sin_full", [128, S])
    din("mask_a", [128, NBLK[0] * 128])
    din("mask_b", [128, NBLK[1] * 128])
    din("ident", [128, 128])
    din("rotT", [128, 128])
    outs = {"out": nc.dram_tensor("out", [2 * CH, HM], FP32,
                                  kind="ExternalOutput").ap()}
    with tile.TileContext(nc) as tc:
        with ExitStack() as ctx:
            build_kernel(ctx, tc, ins, outs)
    nc.compile()
    return nc


def make_in_maps(x, cos, sin, Wqkv, bqkv, Wo):
    xT = np.ascontiguousarray(x.T).astype(bfnp)
    rotT = np.zeros((128, 128), np.float32)
    rotT[64:128, 0:64] = -np.eye(64)
    rotT[0:64, 64:128] = np.eye(64)
    shared = dict(
        xT_full=xT,
        WqT=np.ascontiguousarray(Wqkv[:HM].T).astype(bfnp),
        WkT=np.ascontiguousarray(Wqkv[HM:HM + NKV * D].T).astype(bfnp),
        WvT=np.ascontiguousarray(Wqkv[HM + NKV * D:].T).astype(bfnp),
        WoT=np.ascontiguousarray(Wo.T).astype(bfnp),
        bq=np.ascontiguousarray(
            bqkv[:HM].reshape(KT, 128).T).astype(np.float32),
        bk=np.ascontiguousarray(
            bqkv[HM:HM + NKV * D].reshape(NKV, 128).T).astype(np.float32),
        bv=np.broadcast_to(
            bqkv[HM + NKV * D:], (128, NKV * D)).astype(np.float32).copy(),
        cos_full=np.ascontiguousarray(cos.T).astype(bfnp),
        sin_full=np.ascontiguousarray(sin.T).astype(bfnp),
        ident=np.eye(128, dtype=bfnp),
        rotT=rotT.astype(bfnp),
    )
    qoff = np.arange(CH)[:, None]
    ka = np.arange(NBLK[0] * 128)[None, :]
    kb = np.arange(NBLK[1] * 128)[None, :]
    in_maps = []
    for i in range(NCORE):
        ca, cb = i, NCHUNK - 1 - i
        rows = np.r_[ca * CH:(ca + 1) * CH, cb * CH:(cb + 1) * CH]
        m = dict(shared)
        m["xT_own"] = np.ascontiguousarray(xT[:, rows])
        m["cos_own"] = np.ascontiguousarray(shared["cos_full"][:, rows])
        m["sin_own"] = np.ascontiguousarray(shared["sin_full"][:, rows])
        m["mask_a"] = (ka <= ca * CH + qoff).astype(bfnp)
        m["mask_b"] = (kb <= cb * CH + qoff).astype(bfnp)
        in_maps.append(m)
    return in_maps


def assemble_out(results):
    out = np.zeros((S, HM), np.float32)
    for i, r in enumerate(results):
        o = np.asarray(r["out"], np.float32)
        ca, cb = i, NCHUNK - 1 - i
        out[ca * CH:(ca + 1) * CH] = o[:CH]
        out[cb * CH:(cb + 1) * CH] = o[CH:]
    return out[None]


def kernel(**inputs):
    x = np.asarray(inputs["x"], np.float32)[0]
    cos = np.asarray(inputs["cos"], np.float32)[0]
    sin = np.asarray(inputs["sin"], np.float32)[0]
    Wqkv = np.asarray(inputs["Wqkv"], np.float32)
    bqkv = np.asarray(inputs["bqkv"], np.float32)
    Wo = np.asarray(inputs["Wo"], np.float32)

    if "nc" not in _CACHE:
        _CACHE["nc"] = build_program()
    nc = _CACHE["nc"]
    in_maps = make_in_maps(x, cos, sin, Wqkv, bqkv, Wo)
    res = bass_utils.run_bass_kernel_spmd(nc, in_maps, core_ids=list(range(NCORE)))
    return assemble_out(res.results)
